# revision 13
# baseline (speedup 1.0000x reference)
# Trainium2 Bass kernel for nn_BertAdapter_SLT_49933289783411
#
# Reference computation:
#   y   = tt_linear(x) + bias          (TT-factorized 768->768 linear)
#   out = x + gelu_exact(y)
#
# Key math: the TT cores with ranks [1,5,5,5,5,5,1] factor the 768x768
# weight as W = A @ B with A:(768,5), B:(5,768).  We precompute A,B on
# host (tiny, exact) and run a rank-5 bottleneck matmul on device.
#
# Sharding: data-parallel over the batch dim (8 batch elements -> 8 cores).
# Each core handles x_c:(512,768).  All I/O is bf16 (halves HBM traffic;
# the 2e-2 rel-err budget dwarfs bf16 rounding).  x is pre-transposed on
# host to x^T (feature-major) so the contraction dim lands on SBUF
# partitions.  The 512 rows are processed as 4 quarters of 128 rows, each
# flowing load -> mm1 -> cast -> mm2 -> gelu -> add -> store so the ACT
# engine (the serial bottleneck: ~2.6us of gelu element work) starts as
# early as possible and every stage pipelines across quarters.
#
# Per quarter q (all operands bf16, PSUM accumulation f32):
#   t3_q   = A^T @ x^T_q            (5,128)   PSUM, accumulate over 6 f-chunks
#   y^T_q  = B6^T @ t36_q           (128,768) K=6: B6 rows 0-4 = B, row 5 =
#                                   bias against an all-ones t3 row 5
#   o^T_q  = x^T_q + gelu(y^T_q)    one N=768 gelu op straight from PSUM
#
# B is shipped compact as (6,768) bf16 (9KB) instead of zero-padded to
# K=128 (196KB).  A (128x30 bf16) rides in the head of the x tensor so the
# sync queue issues exactly one DMA per quarter; B goes on the scalar
# (ACT HWDGE) queue; stores alternate sync/gpsimd queues to split the
# ~600ns per-DMA sequencer issue cost.

import numpy as np
import ml_dtypes

import concourse.bass as bass
import concourse.bacc as bacc
import concourse.mybir as mybir
import concourse.tile as tile
from concourse.bass_utils import run_bass_kernel_spmd

HID = 768
ROWS = 512
NPARTS = 4
PSIZE = ROWS // NPARTS      # 128 rows per quarter
NCORES = 8
FCH = 6                     # 768 / 128 feature chunks
RANK = 5
KDIM = RANK + 1             # rank rows + ones row carrying the bias
F32 = mybir.dt.float32
BF16 = mybir.dt.bfloat16

# The HAM grants ONE fixed ~3.4us full-clock boost (k=8/8 -> 2.4GHz) per
# execution, ~2.7us after PE activity becomes sustained; otherwise the PE
# runs at k=4/8 (1.2GHz).  25 warmup matmuls (~2.7us at the throttled
# clock) start the sustain at program start so the boost window lands
# exactly on the real matmul phase (first chunk sem ~10.2us).
N_WARMUP = 27

A_COLS = FCH * RANK                        # 30
XT_COLS = A_COLS + NPARTS * HID            # 30 + 3072

_CACHE = {}


class _LeanTileContext(tile.TileContext):
    """TileContext with a minimal exit sequence.

    The stock exit emits drain + all-engine barrier + per-sem clears +
    barrier (~2-3us).  The runtime re-initializes semaphore state on every
    NEFF execution (verified empirically: repeated executions of the same
    loaded executable stay bit-correct without the clears), so only the
    drain — which makes the kernel end wait for the output DMAs — is kept.
    """

    def _drain_and_barrier(self, tick_clock, wait_clock):
        drain_inst = self.nc.sync.drain()
        wait_clock.add_sem_waits(
            drain_inst.ins, tile.ScopedClock({None: tick_clock.global_clock})
        )
        popped = self.nc._tile_sem_poison_stack.pop()
        assert popped is self._sem_poison


def _build_program(act=None):
    if act is None:
        act = mybir.ActivationFunctionType.Gelu
    nc = bacc.Bacc(None, target_bir_lowering=False)
    xt = nc.dram_tensor("xt", [128, XT_COLS], BF16, kind="ExternalInput")
    bm = nc.dram_tensor("bm", [KDIM, HID], BF16, kind="ExternalInput")
    outt = nc.dram_tensor("outt", [128, NPARTS * HID], BF16, kind="ExternalOutput")

    with _LeanTileContext(nc) as tc:
        with (
            tc.tile_pool(name="const", bufs=1) as cpool,
            tc.tile_pool(name="xs", bufs=1) as xpool,
            tc.tile_pool(name="work", bufs=2) as wpool,
            tc.tile_pool(name="ps_t3", bufs=2, space="PSUM") as tpool,
            tc.tile_pool(name="ps_o", bufs=2, space="PSUM") as opool,
            tc.tile_pool(name="ps_w", bufs=1, space="PSUM") as wps_pool,
        ):
            # B lands on the scalar-engine HWDGE queue so the sync queue's
            # serial ~600ns-per-DMA issue budget is spent on x alone
            bm_sb = cpool.tile([KDIM, HID], BF16)
            nc.scalar.dma_start(bm_sb[:], bm[:])

            x_sb = xpool.tile([128, XT_COLS], BF16)
            a_view = x_sb[:, 0:A_COLS]

            def xq(q, c=0):
                return x_sb[:, A_COLS + q * HID + c * PSIZE : A_COLS + q * HID + (c + 1) * PSIZE]

            # x arrives as 5 chunks spread across two DGE rings: a single
            # ring is descriptor-rate bound at ~185 GB/s; two together reach
            # the ~358 GB/s HBM limit.  The scalar ring is NOT used for x:
            # the Scalar sequencer is busy until ~9.7us with the two
            # ACT_TABLE_LOADs and the bm issue.  q0 is split in half (sync
            # ring, enqueued first) so mm1 q0's first sem fires ~0.5us
            # earlier; per-ring FIFO keeps completions in stream order
            # (sync: q0a,q0b,q2; gpsimd: q1,q3).
            chunks = [
                (0, A_COLS + HID, nc.sync),
                (A_COLS + HID, A_COLS + 2 * HID, nc.gpsimd),
                (A_COLS + 2 * HID, A_COLS + 3 * HID, nc.sync),
                (A_COLS + 3 * HID, A_COLS + 4 * HID, nc.gpsimd),
            ]
            for s, e, dma in chunks:
                dma.dma_start(x_sb[:, s:e], xt[:, s:e])

            # PE warmup: garbage matmuls so the HAM clock gate opens while
            # the x loads are still in flight.  The gate needs ~3.4us of
            # sustained PE activity.  Memsets go on DVE (idle until the
            # first cast) — on gpsimd they would queue behind the q2/q3
            # load issues and delay the warmup (and so the whole PE chain).
            wsb = cpool.tile([128, 128], BF16)
            nc.vector.memset(wsb[:], 0.0)
            wps = wps_pool.tile([128, 128], F32)
            for _ in range(N_WARMUP):
                nc.tensor.matmul(wps[:], wsb[:], wsb[:], start=True, stop=True)

            # rows 0-4 of t3_sb get the per-quarter TT activations; row 5
            # stays at the memset 1.0 and meets the bias row of bm_sb in mm2
            t3_sb = cpool.tile([128, ROWS], BF16)
            nc.vector.memset(t3_sb[:], 1.0)

            # tile_wait_until staggers quarters in the scheduler's sim so
            # the per-engine instruction order matches the stream: the
            # scheduler's DMA cost model otherwise predicts chunk q+1
            # arrives before cast q completes and emits PE order
            # mm1q0,mm1q1,mm2q0 — delaying gelu q0 (the serial ACT chain's
            # start) by ~1.2us.  Floors: quarter q's mm1/cast/mm2/gelu at
            # q; its add+store at q+1.5 so cast q+1 (feeding the PE)
            # precedes add q in the DVE stream.
            for q in range(NPARTS):
                with tc.tile_wait_until(q):
                    t3_ps = tpool.tile([RANK, PSIZE], F32, tag="t3_ps")
                    for c in range(FCH):
                        nc.tensor.matmul(
                            t3_ps[:],
                            a_view[:, c * RANK : (c + 1) * RANK],
                            xq(q, c),
                            start=(c == 0),
                            stop=(c == FCH - 1),
                        )
                    nc.vector.tensor_copy(
                        t3_sb[0:RANK, q * PSIZE : (q + 1) * PSIZE], t3_ps[:]
                    )

                    # (128,1024) f32 = exactly 2 PSUM banks; cols 0-767 used.
                    # start=True on the first matmul touching each bank clears
                    # that bank's has_written bits; later ones overwrite their
                    # still-clear regions.
                    o_ps = opool.tile([128, 1024], F32, tag="o_ps")
                    for j in range(FCH):
                        nc.tensor.matmul(
                            o_ps[:, j * PSIZE : (j + 1) * PSIZE],
                            bm_sb[:, j * PSIZE : (j + 1) * PSIZE],
                            t3_sb[0:KDIM, q * PSIZE : (q + 1) * PSIZE],
                            start=(j in (0, 4)),
                            stop=(j in (3, 5)),
                        )
                xq_full = x_sb[:, A_COLS + q * HID : A_COLS + (q + 1) * HID]
                o_sb = wpool.tile([128, HID], BF16, tag="o_sb", bufs=4)
                g_sb = wpool.tile([128, HID], BF16, tag="g_sb", bufs=3)
                if q < NPARTS - 1:
                    # one N=768 gelu per quarter straight from PSUM amortizes
                    # the ~293ns per-op ACT overhead over the whole quarter
                    with tc.tile_wait_until(q):
                        nc.scalar.activation(g_sb[:], o_ps[:, 0:HID], act, scale=1.0)
                    with tc.tile_wait_until(q + 1.5):
                        nc.vector.tensor_add(o_sb[:], g_sb[:], xq_full)
                        # alternate store rings so consecutive stores'
                        # HBM-write receipts don't queue FIFO behind each
                        # other on one ring; Scalar is avoided (busy with
                        # gelus)
                        dma = nc.gpsimd if q % 2 == 0 else nc.sync
                        dma.dma_start(outt[:, q * HID : (q + 1) * HID], o_sb[:])
                else:
                    # last quarter: gelu+add+store in column halves across
                    # both HWDGE rings so the final store is issued ~0.6us
                    # earlier and its receipt overlaps the second half
                    HH = HID // 2
                    for k, dma in ((0, nc.scalar), (1, nc.sync)):
                        with tc.tile_wait_until(q + k * 0.2):
                            nc.scalar.activation(
                                g_sb[:, k * HH : (k + 1) * HH],
                                o_ps[:, k * HH : (k + 1) * HH],
                                act,
                                scale=1.0,
                            )
                        with tc.tile_wait_until(q + 1.5 + k * 0.2):
                            nc.vector.tensor_add(
                                o_sb[:, k * HH : (k + 1) * HH],
                                g_sb[:, k * HH : (k + 1) * HH],
                                xq_full[:, k * HH : (k + 1) * HH],
                            )
                            dma.dma_start(
                                outt[:, q * HID + k * HH : q * HID + (k + 1) * HH],
                                o_sb[:, k * HH : (k + 1) * HH],
                            )

    nc.finalize()
    return nc


def _get_program():
    if "nc" not in _CACHE:
        _CACHE["nc"] = _build_program()
    return _CACHE["nc"]


def _host_prep(hidden_states, bias, cores):
    """Collapse TT cores to rank-5 factors; pack A + x^T per core in bf16."""
    c0, c1, c2, c3, c4, c5 = [c.astype(np.float64) for c in cores]
    A = np.einsum("iv,vjw,wkx->ijkx", c0[0], c1, c2).reshape(HID, RANK)
    Bm = np.einsum("xpy,yqz,zr->xpqr", c3, c4, c5[:, :, 0]).reshape(RANK, HID)

    a_p = np.ascontiguousarray(
        A.reshape(FCH, 128, RANK).transpose(1, 0, 2).reshape(128, A_COLS)
    ).astype(ml_dtypes.bfloat16)                       # (128, 30)
    bm_p = np.empty((KDIM, HID), dtype=ml_dtypes.bfloat16)
    bm_p[:RANK] = Bm.astype(ml_dtypes.bfloat16)
    bm_p[RANK] = bias.astype(ml_dtypes.bfloat16)       # meets t3_sb's ones row

    xts = []
    for cidx in range(NCORES):
        xct = hidden_states[cidx].T                    # (768, 512) f32
        blocks = [a_p]
        for q in range(NPARTS):
            blocks.append(
                np.ascontiguousarray(xct[:, q * PSIZE : (q + 1) * PSIZE])
                .reshape(FCH, 128, PSIZE)
                .transpose(1, 0, 2)
                .reshape(128, FCH * PSIZE)
                .astype(ml_dtypes.bfloat16)
            )
        xts.append(np.ascontiguousarray(np.concatenate(blocks, axis=1)))
    return xts, bm_p


def _unpack_out(outt_list):
    """outt[p, q*768 + j*128 + r] = out[q*128+r, j*128+p] -> (8, 512, 768)."""
    outs = []
    for outt in outt_list:
        m = np.asarray(outt).reshape(128, NPARTS, FCH, PSIZE)
        o = m.transpose(1, 3, 2, 0).reshape(ROWS, HID)
        outs.append(o)
    return np.stack(outs, axis=0).astype(np.float32)


def run(inputs, trace=False, **spmd_kwargs):
    hidden_states = np.asarray(inputs["hidden_states"], dtype=np.float32)
    bias = np.asarray(inputs["bias"], dtype=np.float32)
    cores = [np.asarray(inputs[f"core{i}"], dtype=np.float32) for i in range(6)]

    xts, bm_p = _host_prep(hidden_states, bias, cores)
    nc = _get_program()
    in_maps = [{"xt": xts[c], "bm": bm_p} for c in range(NCORES)]
    res = run_bass_kernel_spmd(
        nc, in_maps, core_ids=list(range(NCORES)), trace=trace, **spmd_kwargs
    )
    out = _unpack_out([res.results[c]["outt"] for c in range(NCORES)])
    if trace:
        return out, res
    return out


def kernel(**inputs):
    return run(inputs)

